# revision 23
# baseline (speedup 1.0000x reference)
"""Trainium2 Bass kernel for nn_EntityModel (linear head + CRF loss + Viterbi).

kernel(**inputs) takes FULL inputs (as from setup_inputs()), shards the batch
across 8 NeuronCores (data-parallel, no collectives), runs one SPMD Bass
kernel, and returns (loss, predicted_tag) matching the reference.

Per-core (8 batches):
  head:    em^T = (tanh(hidden @ W1 + b1) @ W2 + b2)^T      fp32 PE matmuls
  num:     gold-path score via one-hot bulk reductions       (DVE)
  den:     linear-space forward algorithm a' = (a E) * u, E = exp(trans),
           bidirectional: one block-diag PE matmul + one DVE mult per step
  viterbi: bidirectional max-plus scans (3 DVE ops/dual-step), backpointers
           recomputed in bulk, two gpsimd indirect_copy pointer-chases
"""
import numpy as np

import concourse.bass as bass
import concourse.mybir as mybir
import concourse.tile as tile
from concourse.bass_utils import run_bass_kernel_spmd
from bass_rust import ScopedClock

F32 = mybir.dt.float32
U16 = mybir.dt.uint16
U32 = mybir.dt.uint32
I32 = mybir.dt.int32
OP = mybir.AluOpType
AF = mybir.ActivationFunctionType
AX = mybir.AxisListType

B, S, H, T = 64, 512, 768, 9
BL = 8                  # batches per core
R = BL * S              # rows per core
DELTA = 2.23            # den log-space drift constant
PAD = 9                 # leading pad (elements) on track buffers
NTRK = 257              # tracks 0..256
TW = PAD + NTRK * 9     # 2322
D2 = 2336               # pass-2 bank base inside chaseD

# ---------------------------------------------------------------------------
# workaround: this neuronxcc build allows only ONE sem wait per instruction
# ---------------------------------------------------------------------------
_orig_add = tile.TileContext._add_instruction


def _add_patched(self, inst):
    si = getattr(inst, "sync_info", None)
    if si is not None and si.on_wait is not None and len(si.on_wait) > 1:
        waits = list(si.on_wait)
        for w in waits[:-1]:
            nop = mybir.InstNoOp(name=self.nc.get_next_instruction_name(),
                                 ins=[], outs=[])
            nop.engine = inst.engine
            nop.sync_info = mybir.SyncInfo(on_wait=[w], on_update=[])
            _orig_add(self, nop)
        inst.sync_info = mybir.SyncInfo(on_wait=[waits[-1]],
                                        on_update=list(si.on_update or []))
    _orig_add(self, inst)


def _drain_patched(self, tick_clock, wait_clock):
    nc = self.nc
    drain_inst = nc.sync.drain()
    wait_clock.add_sem_waits(drain_inst.ins,
                             ScopedClock({None: tick_clock.global_clock}))
    si = drain_inst.ins.sync_info
    if si is not None and si.on_wait is not None and len(si.on_wait) > 1:
        waits = list(si.on_wait)
        drain_inst.ins.sync_info = mybir.SyncInfo(
            on_wait=waits[:1], on_update=list(si.on_update or []))
        for w in waits[1:]:
            d2 = nc.sync.drain()
            d2.ins.sync_info = mybir.SyncInfo(on_wait=[w], on_update=[])
    nc.all_engine_barrier()
    popped = nc._tile_sem_poison_stack.pop()
    assert popped is self._sem_poison
    nc.clear_and_free_semaphores(list(self.sems.allocated().values()))
    nc.all_engine_barrier()


tile.TileContext._add_instruction = _add_patched
tile.TileContext._drain_and_barrier = _drain_patched


# ---------------------------------------------------------------------------
# host-side constants (layout/iota constants + re-layouts of tiny inputs)
# ---------------------------------------------------------------------------
def _consts(trans, start, end):
    trans = np.asarray(trans, np.float32)
    c = {}
    c["ident"] = np.eye(128, dtype=np.float32)
    trT_flat = trans.T.reshape(-1)            # trans[i,j] at j*9+i
    tr_flat = trans.reshape(-1)               # trans[i,j] at i*9+j
    tp = np.zeros((16, 81), np.float32)
    tp[0:8] = trT_flat
    tp[8:16] = tr_flat
    c["trtpat"] = tp
    sp = np.zeros((16, 9), np.float32)
    sp[0:8] = np.asarray(start, np.float32)
    sp[8:16] = np.asarray(end, np.float32)
    c["sepat"] = sp
    se2 = np.zeros((8, 18), np.float32)
    se2[:, 0:9] = np.asarray(start, np.float32)
    se2[:, 9:18] = np.asarray(end, np.float32)
    c["sepat2"] = se2
    tp128 = np.zeros((128, 81), np.float32)
    tp128[0:64] = trT_flat
    tp128[64:128] = tr_flat
    c["trpat128"] = tp128
    f81 = np.arange(81)
    c["nmi128"] = np.broadcast_to((9.0 - (f81 % 9)).astype(np.float32),
                                  (128, 81)).copy()
    c["iota81"] = np.broadcast_to(f81.astype(np.float32), (128, 81)).copy()
    c["iota9"] = np.broadcast_to(np.arange(9, dtype=np.float32),
                                 (128, 9)).copy()
    c["trf_rep"] = np.broadcast_to(tr_flat, (128, 81)).copy()
    c["trfpat8"] = np.broadcast_to(tr_flat, (8, 81)).copy()
    # bank value ramp: 9*c at (p, s'*9+j), c = (p%8)*32 + s'; clamp c==0
    ramp = np.zeros((128, 288), np.float32)
    sc = (np.arange(128) % 8)[:, None]
    cidx = sc * 32 + (np.arange(288) // 9)[None, :]
    ramp[:] = 9.0 * cidx
    ramp[cidx == 0] = 20.0
    c["ramp128"] = ramp
    # chase-tag ramp on partitions 0::16: col t<=256: (255-t)*9 ; col 257+u: (256-u)*9
    rt = np.zeros((128, 514), np.float32)
    t = np.arange(257, dtype=np.float32)
    rt[0::16, 0:257] = (255.0 - t) * 9.0
    rt[0::16, 257:514] = (256.0 - t) * 9.0
    c["rampt"] = rt
    m0 = np.zeros((128, 32), np.float32)
    m0[0::16, 0] = 100.0
    c["m0fix"] = m0
    grp = np.zeros((128, 8), np.float32)
    grp[np.arange(128), np.arange(128) // 16] = 1.0
    c["grp"] = grp
    c["ones9"] = np.ones((9, 1), np.float32)
    blk = np.full((18, 18), -200.0, np.float32)
    blk[0:9, 0:9] = trans
    blk[9:18, 9:18] = trans.T
    c["trblk"] = blk
    return c


_CONST_SHAPES = {
    "ident": [128, 128], "trtpat": [16, 81], "sepat": [16, 9],
    "sepat2": [8, 18], "trpat128": [128, 81], "nmi128": [128, 81],
    "iota81": [128, 81], "iota9": [128, 9], "trf_rep": [128, 81],
    "trfpat8": [8, 81],
    "ramp128": [128, 288], "rampt": [128, 514], "m0fix": [128, 32],
    "grp": [128, 8], "ones9": [9, 1], "trblk": [18, 18],
}


# ---------------------------------------------------------------------------
# kernel build
# ---------------------------------------------------------------------------
def build_nc(debug=False):
    nc = bass.Bass()
    hid = nc.declare_dram_parameter("hid", [R, H], F32, isOutput=False)
    w1 = nc.declare_dram_parameter("w1", [H, H], F32, isOutput=False)
    b1c = nc.declare_dram_parameter("b1c", [H, 1], F32, isOutput=False)
    w2 = nc.declare_dram_parameter("w2", [H, T], F32, isOutput=False)
    b2c = nc.declare_dram_parameter("b2c", [T, 1], F32, isOutput=False)
    startc = nc.declare_dram_parameter("startc", [T, 1], F32, isOutput=False)
    endc = nc.declare_dram_parameter("endc", [T, 1], F32, isOutput=False)
    tagsf = nc.declare_dram_parameter("tagsf", [BL, S + 1], F32, isOutput=False)
    cin = {k: nc.declare_dram_parameter(k, shp, F32, isOutput=False)
           for k, shp in _CONST_SHAPES.items()}
    llh_out = nc.declare_dram_parameter("llh_out", [BL, 1], F32, isOutput=True)
    tag_out = nc.declare_dram_parameter("tag_out", [BL, S], I32, isOutput=True)
    dram_em = nc.dram_tensor("dram_em", [T, R], F32)
    dram_em2 = nc.dram_tensor("dram_em2", [R, T], F32)
    dbg = {}
    if debug:
        dbg["dbg_emT"] = nc.declare_dram_parameter("dbg_emT", [T, R+2], F32, isOutput=True)
        dbg["dbg_hist"] = nc.declare_dram_parameter("dbg_hist", [16, TW], F32, isOutput=True)
        dbg["dbg_meetv"] = nc.declare_dram_parameter("dbg_meetv", [BL, 9], F32, isOutput=True)
        dbg["dbg_mi8"] = nc.declare_dram_parameter("dbg_mi8", [BL, 8], U32, isOutput=True)
        dbg["dbg_chaseH"] = nc.declare_dram_parameter("dbg_chaseH", [128, 2056], U32, isOutput=True)
        dbg["dbg_rawp"] = nc.declare_dram_parameter("dbg_rawp", [128, 288], F32, isOutput=True)
        dbg["dbg_bank"] = nc.declare_dram_parameter("dbg_bank", [128, 288], U32, isOutput=True)
        dbg["dbg_tagv"] = nc.declare_dram_parameter("dbg_tagv", [128, 514], F32, isOutput=True)

    with tile.TileContext(nc) as tc:
        with tc.tile_pool(name="persist", bufs=1) as pp:
            emT = pp.tile([T, R + 2], F32)        # data cols 1..4096
            hist = pp.tile([16, TW], F32)         # fwd rows 0-7, bwd 8-15
            em_b2 = pp.tile([16, TW], F32)        # track-layout emissions
            nsc_hist = pp.tile([16, TW], F32)     # pre-emission reduce outputs
            U2 = pp.tile([18, 2048], F32)         # den u factors, col t*8+b
            E2 = pp.tile([18, 18], F32)
            chaseD = pp.tile([128, 4672], U32)    # banks at partitions 0::16
            chaseH = pp.tile([128, 2056], U32)
            tagsf_sb = pp.tile([BL, S + 1], F32)
            csb = {k: pp.tile(shp, F32, name=f"c_{k}") for k, shp in _CONST_SHAPES.items()}
            for k in _CONST_SHAPES:
                nc.sync.dma_start(out=csb[k][:], in_=cin[k][:])
            nc.sync.dma_start(out=tagsf_sb[:], in_=tagsf[:])
            nc.scalar.activation(E2[:], csb["trblk"][:], AF.Exp)

            # =============== head ===============
            with (
                tc.tile_pool(name="hd", bufs=1) as hp,
                tc.tile_pool(name="hrow", bufs=4) as hrp,
                tc.tile_pool(name="hidt", bufs=12) as htp,
                tc.tile_pool(name="hT", bufs=12) as hTp,
                tc.tile_pool(name="psT", bufs=2, space="PSUM") as psTp,
                tc.tile_pool(name="ps1", bufs=2, space="PSUM") as ps1p,
                tc.tile_pool(name="ps2", bufs=2, space="PSUM") as ps2p,
            ):
                w1sb = [hp.tile([128, H], F32, name=f"w1sb{k}") for k in range(6)]
                w2sb = [hp.tile([128, T], F32, name=f"w2sb{k}") for k in range(6)]
                b1sb = [hp.tile([128, 1], F32, name=f"b1sb{k}") for k in range(6)]
                for k in range(6):
                    nc.sync.dma_start(out=w1sb[k][:], in_=w1[k*128:(k+1)*128, :])
                    nc.sync.dma_start(out=w2sb[k][:], in_=w2[k*128:(k+1)*128, :])
                    nc.sync.dma_start(out=b1sb[k][:], in_=b1c[k*128:(k+1)*128, :])
                b2sb = hp.tile([T, 1], F32)
                nc.sync.dma_start(out=b2sb[:], in_=b2c[:])

                for g in range(8):
                    hrows = []
                    for rt in range(4):
                        hr = hrp.tile([128, H], F32, tag="hrow")
                        nc.sync.dma_start(
                            out=hr[:],
                            in_=hid[g*512 + rt*128: g*512 + (rt+1)*128, :])
                        hrows.append(hr)
                    hts = []
                    for k in range(6):
                        pst = psTp.tile([128, 512], F32, tag="psT")
                        for rt in range(4):
                            nc.tensor.transpose(
                                pst[:, rt*128:(rt+1)*128],
                                hrows[rt][:, k*128:(k+1)*128],
                                csb["ident"][:])
                        ht = htp.tile([128, 512], F32, tag="hidt")
                        nc.vector.tensor_copy(ht[:], pst[:])
                        hts.append(ht)
                    p2 = ps2p.tile([T, 512], F32, tag="ps2")
                    for o in range(6):
                        p1 = ps1p.tile([128, 512], F32, tag="ps1")
                        for k in range(6):
                            nc.tensor.matmul(
                                p1[:], w1sb[k][:, o*128:(o+1)*128], hts[k][:],
                                start=(k == 0), stop=(k == 5))
                        hT = hTp.tile([128, 512], F32, tag="hT")
                        nc.scalar.activation(hT[:], p1[:], AF.Tanh,
                                             bias=b1sb[o][:, 0:1])
                        nc.tensor.matmul(p2[:], w2sb[o][:], hT[:],
                                         start=(o == 0), stop=(o == 5))
                    nc.scalar.activation(emT[:, 1+g*512: 1+(g+1)*512], p2[:],
                                         AF.Identity, bias=b2sb[:, 0:1])

            # =============== CRF phase ===============
            with (
                tc.tile_pool(name="crf", bufs=1) as cp,
                tc.tile_pool(name="big", bufs=2) as bigp,
                tc.tile_pool(name="sc16", bufs=2) as scp,
                tc.tile_pool(name="st2", bufs=2) as stp,
                tc.tile_pool(name="dps", bufs=2, space="PSUM") as dpsp,
                tc.tile_pool(name="sps", bufs=2, space="PSUM") as spsp,
            ):
                emT_bs = emT[:, 1:R+1].rearrange("p (b s) -> p b s", b=BL)
                # bounce em^T through DRAM for partition-restructuring repacks
                nc.sync.dma_start(out=dram_em[:], in_=emT[:, 1:R+1])
                # j-fastest copy: dram_em2[r, j] (transposing write AP)
                nc.sync.dma_start(
                    out=dram_em2[:].transpose([1, 0]), in_=emT[:, 1:R+1])
                # ---- em track layout: fwd col PAD+c*9+j = em[b,c,j];
                #      bwd = em[b,511-c,j], c = 0..256 ----
                nc.sync.dma_start(
                    out=em_b2[0:8, PAD: PAD+NTRK*9],
                    in_=dram_em2[:].rearrange("(b s) j -> b (s j)", b=BL)
                        [:, 0:NTRK*9])
                for j in range(T):
                    ej = dram_em[j:j+1, :].rearrange("p (b s) -> (p b) s", b=BL)
                    nc.sync.dma_start(out=em_b2[8:16, PAD+j::9],
                                      in_=ej[:, 511:254:-1])
                # em_bulk for num: [(b,c16), (s', j)] -- per-chunk 2D DMAs
                em_bulk = cp.tile([128, 288], F32)
                dram_em2f = dram_em2[:].rearrange("(b s) j -> b (s j)", b=BL)
                for cch in range(16):
                    nc.sync.dma_start(
                        out=em_bulk[cch::16, :],
                        in_=dram_em2f[:, cch*288:(cch+1)*288])

                # ---- num ----
                tcur = cp.tile([128, 32], F32)
                tprv = cp.tile([128, 32], F32)
                for cch in range(16):
                    nc.sync.dma_start(out=tcur[cch::16, :],
                                      in_=tagsf[:, 1+cch*32: 1+(cch+1)*32])
                    nc.sync.dma_start(out=tprv[cch::16, :],
                                      in_=tagsf[:, cch*32: (cch+1)*32])
                eq9 = cp.tile([128, 288], F32)
                nc.vector.tensor_tensor(
                    out=eq9[:].rearrange("p (s j) -> p s j", j=9),
                    in0=tcur[:].unsqueeze(2).broadcast_to([128, 32, 9]),
                    in1=csb["iota9"][:].unsqueeze(1).broadcast_to([128, 32, 9]),
                    op=OP.is_equal)
                scr288 = cp.tile([128, 288], F32)
                sum_em = cp.tile([128, 1], F32)
                nc.vector.tensor_tensor(out=scr288[:], in0=eq9[:],
                                        in1=em_bulk[:], op=OP.mult)
                nc.vector.tensor_reduce(out=sum_em[:], in_=scr288[:],
                                        axis=AX.X, op=OP.add)
                pidx = cp.tile([128, 32], F32)
                nc.vector.scalar_tensor_tensor(
                    out=pidx[:], in0=tprv[:], scalar=9.0, in1=tcur[:],
                    op0=OP.mult, op1=OP.add)
                nc.vector.tensor_tensor(out=pidx[:], in0=pidx[:],
                                        in1=csb["m0fix"][:], op=OP.add)
                eq81 = bigp.tile([128, 2592], F32, tag="big")
                nc.vector.tensor_tensor(
                    out=eq81[:].rearrange("p (s k) -> p s k", k=81),
                    in0=pidx[:].unsqueeze(2).broadcast_to([128, 32, 81]),
                    in1=csb["iota81"][:].unsqueeze(1).broadcast_to([128, 32, 81]),
                    op=OP.is_equal)
                scr2592 = bigp.tile([128, 2592], F32, tag="big")
                sum_tr = cp.tile([128, 1], F32)
                nc.vector.tensor_tensor(
                    out=scr2592[:].rearrange("p (s k) -> p s k", k=81),
                    in0=eq81[:].rearrange("p (s k) -> p s k", k=81),
                    in1=csb["trf_rep"][:].unsqueeze(1).broadcast_to([128, 32, 81]),
                    op=OP.mult)
                nc.vector.tensor_reduce(out=sum_tr[:], in_=scr2592[:],
                                        axis=AX.X, op=OP.add)
                sums = cp.tile([128, 1], F32)
                nc.vector.tensor_tensor(out=sums[:], in0=sum_em[:],
                                        in1=sum_tr[:], op=OP.add)
                num8 = spsp.tile([BL, 1], F32, tag="spsA")
                nc.tensor.matmul(num8[:], csb["grp"][:], sums[:],
                                 start=True, stop=True)
                eqs = cp.tile([BL, 9], F32)
                eqe = cp.tile([BL, 9], F32)
                nc.vector.tensor_tensor(
                    out=eqs[:], in0=tagsf_sb[:, 1:2].broadcast_to([BL, 9]),
                    in1=csb["iota9"][0:BL, :], op=OP.is_equal)
                nc.vector.tensor_tensor(
                    out=eqe[:], in0=tagsf_sb[:, S:S+1].broadcast_to([BL, 9]),
                    in1=csb["iota9"][0:BL, :], op=OP.is_equal)
                scrse = cp.tile([BL, 9], F32)
                scrse2 = cp.tile([BL, 9], F32)
                st_t = cp.tile([BL, 1], F32)
                en_t = cp.tile([BL, 1], F32)
                nc.vector.tensor_tensor(out=scrse[:], in0=eqs[:],
                                        in1=csb["sepat2"][:, 0:9], op=OP.mult)
                nc.vector.tensor_reduce(out=st_t[:], in_=scrse[:],
                                        axis=AX.X, op=OP.add)
                nc.vector.tensor_tensor(out=scrse2[:], in0=eqe[:],
                                        in1=csb["sepat2"][:, 9:18], op=OP.mult)
                nc.vector.tensor_reduce(out=en_t[:], in_=scrse2[:],
                                        axis=AX.X, op=OP.add)

                # ---- den ----
                emT_sb = emT_bs.transpose([0, 2, 1])      # [9, 512, 8]
                negd = cp.tile([18, 1], F32)
                nc.vector.memset(negd[:], -DELTA)
                nc.scalar.activation(
                    U2[0:9, 8:2048].rearrange("p (t b) -> p t b", b=8),
                    emT_sb[:, 1:256, :], AF.Exp, bias=negd[0:9, 0:1])
                # bwd u's computed on partitions 0..8, then DMA-moved to 9..17
                U2b = cp.tile([T, 2040], F32)
                nc.scalar.activation(
                    U2b[:].rearrange("p (t b) -> p t b", b=8),
                    emT_sb[:, 510:255:-1, :], AF.Exp, bias=negd[0:9, 0:1])
                nc.sync.dma_start(out=U2[9:18, 8:2048], in_=U2b[:])
                scrA = cp.tile([18, 8], F32)
                bias2 = cp.tile([18, 1], F32)
                nc.sync.dma_start(out=scrA[0:9, :], in_=emT_sb[:, 0, :])
                nc.sync.dma_start(out=scrA[9:18, :], in_=emT_sb[:, 511, :])
                nc.sync.dma_start(out=bias2[0:9, :], in_=startc[:])
                nc.sync.dma_start(out=bias2[9:18, :], in_=endc[:])
                nc.vector.tensor_scalar_sub(bias2[:], bias2[:], DELTA)
                st0 = stp.tile([18, 8], F32, tag="st2")
                nc.scalar.activation(st0[:], scrA[:], AF.Exp, bias=bias2[:, 0:1])
                sprev = st0
                for t in range(1, 256):
                    dps = dpsp.tile([18, 8], F32, tag="dps")
                    nc.tensor.matmul(dps[:], E2[:], sprev[:],
                                     start=True, stop=True)
                    snew = stp.tile([18, 8], F32, tag="st2")
                    nc.vector.tensor_tensor(out=snew[:], in0=dps[:],
                                            in1=U2[:, t*8:(t+1)*8], op=OP.mult)
                    sprev = snew
                dfin = dpsp.tile([18, 8], F32, tag="dps")
                nc.tensor.matmul(dfin[:], E2[:], sprev[:], start=True, stop=True)
                wmv = cp.tile([9, 8], F32)
                nc.sync.dma_start(out=wmv[:], in_=sprev[9:18, :])
                dprod = cp.tile([9, 8], F32)
                nc.vector.tensor_tensor(out=dprod[:], in0=dfin[0:9, :],
                                        in1=wmv[:], op=OP.mult)
                den8 = spsp.tile([BL, 1], F32, tag="spsB")
                nc.tensor.matmul(den8[:], dprod[:], csb["ones9"][:],
                                 start=True, stop=True)
                lnden = cp.tile([BL, 1], F32)
                nc.scalar.activation(lnden[:], den8[:], AF.Ln)
                llh = cp.tile([BL, 1], F32)
                nc.vector.tensor_tensor(out=llh[:], in0=st_t[:], in1=en_t[:],
                                        op=OP.add)
                nc.vector.tensor_tensor(out=llh[:], in0=llh[:], in1=num8[:],
                                        op=OP.add)
                nc.vector.tensor_tensor(out=llh[:], in0=llh[:], in1=lnden[:],
                                        op=OP.subtract)
                nc.vector.tensor_scalar_sub(llh[:], llh[:], 512.0 * DELTA)
                nc.sync.dma_start(out=llh_out[:], in_=llh[:])

                # ---- viterbi bidirectional scan ----
                trt_v = csb["trtpat"][0:16, :].rearrange("p (a b) -> p a b", b=9)
                nc.vector.tensor_tensor(out=hist[:, PAD:PAD+9],
                                        in0=csb["sepat"][0:16, :],
                                        in1=em_b2[:, PAD:PAD+9], op=OP.add)
                for t in range(1, 257):
                    cand = scp.tile([16, 81], F32, tag="cand")
                    nc.vector.tensor_tensor(
                        out=cand[:].rearrange("p (a b) -> p a b", b=9),
                        in0=hist[:, PAD+(t-1)*9: PAD+t*9]
                            .unsqueeze(1).broadcast_to([16, 9, 9]),
                        in1=trt_v, op=OP.add)
                    nc.vector.tensor_reduce(
                        out=nsc_hist[:, PAD+t*9: PAD+(t+1)*9],
                        in_=cand[:].rearrange("p (a b) -> p a b", b=9),
                        axis=AX.X, op=OP.max)
                    nc.vector.tensor_tensor(
                        out=hist[:, PAD+t*9: PAD+(t+1)*9],
                        in0=nsc_hist[:, PAD+t*9: PAD+(t+1)*9],
                        in1=em_b2[:, PAD+t*9: PAD+(t+1)*9], op=OP.add)

                # ---- meet at s=255 ----
                bmv = cp.tile([BL, 9], F32)
                nc.sync.dma_start(out=bmv[:],
                                  in_=nsc_hist[8:16, PAD+256*9: PAD+257*9])
                meetv = cp.tile([BL, 9], F32)
                nc.vector.tensor_tensor(out=meetv[:],
                                        in0=hist[0:8, PAD+255*9: PAD+256*9],
                                        in1=bmv[:], op=OP.add)
                mx8 = cp.tile([BL, 8], F32)
                mi8 = cp.tile([BL, 8], U32)
                nc.vector.max(mx8[:], meetv[:])
                nc.vector.max_index(mi8[:], mx8[:], meetv[:])
                mtf = cp.tile([BL, 1], F32)
                nc.vector.tensor_copy(mtf[:], mi8[:, 0:1])
                initf = cp.tile([BL, 1], U32)
                initb = cp.tile([BL, 1], U32)
                nc.vector.tensor_scalar_add(initf[:], mtf[:], 255.0 * 9.0)
                nc.vector.tensor_scalar_add(initb[:], mtf[:], 256.0 * 9.0)

                # ---- bulk backpointer recompute (both dirs, one pass) ----
                histb = cp.tile([128, 297], F32)
                sm = cp.tile([128, 297], F32)
                for sc in range(8):
                    nc.sync.dma_start(out=histb[sc::8, :],
                                      in_=hist[:, sc*288: sc*288+297])
                    nc.sync.dma_start(out=sm[sc::8, :],
                                      in_=nsc_hist[:, sc*288: sc*288+297])
                candb = bigp.tile([128, 2592], F32, tag="big")
                nc.vector.tensor_tensor(
                    out=candb[:].rearrange("p (s a b) -> p s a b", a=9, b=9),
                    in0=histb[:, 0:288].rearrange("p (s j) -> p s j", j=9)
                        .unsqueeze(2).broadcast_to([128, 32, 9, 9]),
                    in1=csb["trpat128"][:].unsqueeze(1)
                        .broadcast_to([128, 32, 81])
                        .rearrange("p s (a b) -> p s a b", b=9),
                    op=OP.add)
                eqb = bigp.tile([128, 2592], F32, tag="big")
                nc.vector.tensor_tensor(
                    out=eqb[:].rearrange("p (s a b) -> p s a b", a=9, b=9),
                    in0=candb[:].rearrange("p (s a b) -> p s a b", a=9, b=9),
                    in1=sm[:, 9:297].rearrange("p (s j) -> p s j", j=9)
                        .unsqueeze(3).broadcast_to([128, 32, 9, 9]),
                    op=OP.is_equal)
                selb = bigp.tile([128, 2592], F32, tag="big")
                nc.vector.tensor_tensor(
                    out=selb[:].rearrange("p (s a b) -> p s a b", a=9, b=9),
                    in0=eqb[:].rearrange("p (s a b) -> p s a b", a=9, b=9),
                    in1=csb["nmi128"][:].unsqueeze(1)
                        .broadcast_to([128, 32, 81])
                        .rearrange("p s (a b) -> p s a b", b=9),
                    op=OP.mult)
                rawp = cp.tile([128, 288], F32)
                nc.vector.tensor_reduce(
                    out=rawp[:].rearrange("p (s j) -> p s j", j=9),
                    in_=selb[:].rearrange("p (s a b) -> p s a b", a=9, b=9),
                    axis=AX.X, op=OP.max)
                bank16 = cp.tile([128, 288], U32)
                nc.vector.tensor_tensor(out=bank16[:], in0=csb["ramp128"][:],
                                        in1=rawp[:], op=OP.subtract)
                nc.vector.memset(chaseD[:], 0)
                for sc in range(8):
                    nc.sync.dma_start(
                        out=chaseD[0::16, sc*288:(sc+1)*288],
                        in_=bank16[sc:64:8, :])
                    nc.sync.dma_start(
                        out=chaseD[0::16, D2+sc*288: D2+(sc+1)*288],
                        in_=bank16[64+sc:128:8, :])

                # extra bwd bank entries for track c=256 (moved to partitions 0..7)
                bw255 = cp.tile([BL, 9], F32)
                nc.sync.dma_start(out=bw255[:],
                                  in_=hist[8:16, PAD+255*9: PAD+256*9])
                c256 = cp.tile([BL, 81], F32)
                nc.vector.tensor_tensor(
                    out=c256[:].rearrange("p (a b) -> p a b", b=9),
                    in0=bw255[:].unsqueeze(1).broadcast_to([8, 9, 9]),
                    in1=csb["trfpat8"][:].rearrange("p (a b) -> p a b", b=9),
                    op=OP.add)
                sm256 = cp.tile([BL, 9], F32)
                nc.sync.dma_start(out=sm256[:],
                                  in_=nsc_hist[8:16, PAD+256*9: PAD+257*9])
                eq256 = cp.tile([BL, 81], F32)
                nc.vector.tensor_tensor(
                    out=eq256[:].rearrange("p (a b) -> p a b", b=9),
                    in0=c256[:].rearrange("p (a b) -> p a b", b=9),
                    in1=sm256[:].unsqueeze(2).broadcast_to([8, 9, 9]),
                    op=OP.is_equal)
                sel256 = cp.tile([BL, 81], F32)
                nc.vector.tensor_tensor(
                    out=sel256[:].rearrange("p (a b) -> p a b", b=9),
                    in0=eq256[:].rearrange("p (a b) -> p a b", b=9),
                    in1=csb["nmi128"][0:BL, :].rearrange("p (a b) -> p a b",
                                                         b=9),
                    op=OP.mult)
                raw256 = cp.tile([BL, 9], F32)
                nc.vector.tensor_reduce(
                    out=raw256[:],
                    in_=sel256[:].rearrange("p (a b) -> p a b", b=9),
                    axis=AX.X, op=OP.max)
                bank256 = cp.tile([BL, 9], U32)
                nc.vector.tensor_scalar(
                    out=bank256[:], in0=raw256[:], scalar1=-1.0,
                    scalar2=2304.0, op0=OP.mult, op1=OP.add)
                nc.sync.dma_start(out=chaseD[0::16, D2+2304: D2+2313],
                                  in_=bank256[:])

                # ---- two pointer-chases (u32 banks, 4-wide gathers;
                #      only lane 0 of each 16-partition group is real) ----
                chaseHu = chaseH[:].bitcast(U16)      # [128, 2*2056]
                nc.vector.memset(chaseH[:], 0)
                nc.sync.dma_start(out=chaseH[0::16, 0:1], in_=initf[:])
                nc.sync.dma_start(out=chaseH[0::16, 1028:1029], in_=initb[:])
                for t in range(1, 256):
                    nc.gpsimd.indirect_copy(
                        chaseH[:, 4*t: 4*t+4], chaseD[:, 0:2322],
                        chaseHu[:, 8*(t-1): 8*(t-1)+1], True)
                for t in range(1, 257):
                    nc.gpsimd.indirect_copy(
                        chaseH[:, 1028+4*t: 1028+4*t+4], chaseD[:, D2:D2+2322],
                        chaseHu[:, 2056+8*(t-1): 2056+8*(t-1)+1], True)

                # ---- tag assembly ----
                tagv = cp.tile([128, 514], F32)
                nc.vector.tensor_copy(tagv[:], chaseH[:, 0::4])
                nc.vector.tensor_tensor(out=tagv[:], in0=tagv[:],
                                        in1=csb["rampt"][:], op=OP.subtract)
                tags8 = cp.tile([BL, S], F32)
                nc.sync.dma_start(out=tags8[:, 0:256],
                                  in_=tagv[0::16, 255::-1])
                nc.sync.dma_start(out=tags8[:, 256:512],
                                  in_=tagv[0::16, 258:514])
                tagsi = cp.tile([BL, S], I32)
                nc.vector.tensor_copy(tagsi[:], tags8[:])
                nc.sync.dma_start(out=tag_out[:], in_=tagsi[:])
                if debug:
                    nc.sync.dma_start(out=dbg["dbg_emT"][:], in_=emT[:])
                    nc.sync.dma_start(out=dbg["dbg_hist"][:], in_=hist[:])
                    nc.sync.dma_start(out=dbg["dbg_meetv"][:], in_=meetv[:])
                    nc.sync.dma_start(out=dbg["dbg_mi8"][:], in_=mi8[:])
                    nc.sync.dma_start(out=dbg["dbg_chaseH"][:], in_=chaseH[:])
                    nc.sync.dma_start(out=dbg["dbg_rawp"][:], in_=rawp[:])
                    nc.sync.dma_start(out=dbg["dbg_bank"][:], in_=bank16[:])
                    nc.sync.dma_start(out=dbg["dbg_tagv"][:], in_=tagv[:])

    return nc


# ---------------------------------------------------------------------------
# host wrapper
# ---------------------------------------------------------------------------
_NC_CACHE = {}


def _get_nc():
    if "nc" not in _NC_CACHE:
        _NC_CACHE["nc"] = build_nc()
    return _NC_CACHE["nc"]


def _run(inputs, trace=False):
    hidden = np.ascontiguousarray(np.asarray(inputs["hidden"], np.float32))
    mask = np.asarray(inputs["mask"])
    assert mask.min() >= 1, "kernel specialized for all-ones mask"
    tags = np.asarray(inputs["target_tag"]).astype(np.int64)
    w1 = np.ascontiguousarray(np.asarray(inputs["W_fcl"], np.float32))
    b1 = np.asarray(inputs["b_fcl"], np.float32).reshape(H, 1)
    w2 = np.ascontiguousarray(np.asarray(inputs["W_tag"], np.float32))
    b2 = np.asarray(inputs["b_tag"], np.float32).reshape(T, 1)
    start = np.asarray(inputs["start_trans"], np.float32).reshape(T, 1)
    end = np.asarray(inputs["end_trans"], np.float32).reshape(T, 1)
    trans = np.ascontiguousarray(np.asarray(inputs["trans"], np.float32))

    consts = _consts(trans, start[:, 0], end[:, 0])
    nc = _get_nc()
    in_maps = []
    for c in range(8):
        tf = np.zeros((BL, S + 1), np.float32)
        tsl = tags[c*BL:(c+1)*BL].astype(np.float32)
        tf[:, 1:] = tsl
        tf[:, 0] = tsl[:, 0]
        m = {
            "hid": np.ascontiguousarray(hidden[c*BL:(c+1)*BL].reshape(R, H)),
            "w1": w1, "b1c": b1, "w2": w2, "b2c": b2,
            "startc": start, "endc": end, "tagsf": tf,
        }
        m.update(consts)
        in_maps.append(m)
    res = run_bass_kernel_spmd(nc, in_maps, list(range(8)), trace=trace)
    llh = np.concatenate([res.results[c]["llh_out"][:, 0] for c in range(8)])
    tags_out = np.concatenate([res.results[c]["tag_out"] for c in range(8)])
    loss = np.float32(-(llh.sum() / B))
    return (loss, tags_out.astype(np.int32)), res


def kernel(**inputs):
    (loss, tags_out), _ = _run(inputs, trace=False)
    return loss, tags_out
